# revision 19
# baseline (speedup 1.0000x reference)
"""LmHead (RMSNorm -> vocab projection -> top-1 token) on 8 trn2 NeuronCores.

Sharding: lm_head_weight is split over the vocab dim (4000 rows per core,
tensor-parallel).  Each core streams its weight shard from HBM, computes
local logits for all 8 batch rows on the PE, and reduces them to local
top-8 candidate sets with the DVE Max8 unit.  The host then combines the
per-core candidates into the global argmax.

The kernel is memory-bound (weight streaming dominates), so the shard is
prepared host-side in the exact SBUF image the kernel wants:
  - transposed so the contraction dim D lands on SBUF partitions
    (16 KB-contiguous DMA descriptors, full HBM rate), and
  - cast to fp16, halving the bytes streamed and making each PE matmul a
    single pass (fp32 matmuls lower to two PE passes on trn2).
fp16 logits are a prescreen only: each core keeps the top-8 of every
250-column block (fp16 noise is ~5e-4 vs ~0.1 typical top-2 gaps, so the
true winner is always captured), and the host rescores all candidates
against the fp32 weights in float64 to pick the exact argmax.
"""

import os
import sys
import types

import numpy as np

B = 8
D = 4096
V = 32000
NCORES = 8
VS = V // NCORES  # 4000 vocab rows per core
P = 128
T = D // P  # 32 contraction chunks
NVB = 16  # vocab blocks per core
VBLK = VS // NVB  # 250 columns per block
K8 = 8  # Max8 width

DEFAULT_MODE = os.environ.get("LMHEAD_MODE", "fp16")

_STATE = {}


def _ensure_profile_hook():
    """Register the axon NTFF profiling hook if the image's antenv lacks it.

    Harmless when tracing is never requested; lets test.py pass trace=True.
    """
    if "antenv.axon_hooks" in sys.modules:
        return
    try:
        import antenv  # noqa: F401
        from trn_agent_boot.trn_boot import _ntff_profile_via_ctypes

        hook = _ntff_profile_via_ctypes("/opt/axon/libaxon_pjrt.so")
        mod = types.ModuleType("antenv.axon_hooks")
        mod.get_axon_ntff_profile_hook = lambda: hook
        mod.set_axon_ntff_profile_hook = lambda h: None
        sys.modules["antenv.axon_hooks"] = mod
    except Exception:
        pass


def _build_prescreen(wdt_name):
    """Reduced-precision prescreen kernel: per-block top-8 indices for host
    rescoring.  wdt_name: 'float16' or 'float8e4'."""
    from concourse import bacc
    import concourse.mybir as mybir
    from concourse.tile import TileContext
    from concourse.masks import make_identity

    f32 = mybir.dt.float32
    f16 = getattr(mybir.dt, wdt_name)
    nc = bacc.Bacc("TRN2", debug=False, num_devices=NCORES)
    # host layout: wt[vb, p, t, v] = W_shard[vb*VBLK + v, t*P + p], fp16/fp8
    wt = nc.dram_tensor("wt", [NVB, P, T * VBLK], f16, kind="ExternalInput")
    # host layout: xt[p, t, b] = x[b, t*P + p] (pure layout prep, no arithmetic)
    xt_d = nc.dram_tensor("xt", [P, T * B], f32, kind="ExternalInput")
    gt_d = nc.dram_tensor("gt", [P, T], f32, kind="ExternalInput")
    outi = nc.dram_tensor("outi", [B, NVB * K8], mybir.dt.uint32, kind="ExternalOutput")

    with TileContext(nc) as tc:
        with (
            tc.tile_pool(name="const", bufs=1) as cpool,
            tc.tile_pool(name="wpool", bufs=8) as wpool,
            tc.tile_pool(name="psacc", bufs=3, space="PSUM") as psacc,
        ):
            # --- Phase 0: hT[d, (t,b)] = cast(xT[d, (t,b)] * gamma[d-chunk t]) ---
            xt = cpool.tile([P, T * B], f32)
            nc.gpsimd.dma_start(out=xt[:, :], in_=xt_d.ap())
            gt = cpool.tile([P, T], f32)
            nc.gpsimd.dma_start(out=gt[:, :], in_=gt_d.ap())
            hT = cpool.tile([P, T * B], f16)
            for t in range(T):
                nc.vector.tensor_scalar_mul(
                    hT[:, t * B : (t + 1) * B],
                    xt[:, t * B : (t + 1) * B],
                    gt[:, t : t + 1],
                )

            # --- Phase 1: per vocab block, stream weights + matmul + local top-8 ---
            scratch = cpool.tile([B, NVB * K8], f32)  # per-block top-8 values
            idxs = cpool.tile([B, NVB * K8], mybir.dt.uint32)
            lg = cpool.tile([B, NVB * VBLK], f32)  # block logits (SBUF, for Max8)
            TH = T // 2
            for vb in range(NVB):
                w = wpool.tile([P, T * VBLK], f16)
                # split per-block stream across both HWDGE rings
                nc.sync.dma_start(
                    out=w[:, : TH * VBLK], in_=wt.ap()[vb, :, : TH * VBLK]
                )
                nc.scalar.dma_start(
                    out=w[:, TH * VBLK :], in_=wt.ap()[vb, :, TH * VBLK :]
                )
                acc = psacc.tile([B, VBLK], f32)
                for t in range(T):
                    nc.tensor.matmul(
                        acc[:, :],
                        lhsT=hT[:, t * B : (t + 1) * B],
                        rhs=w[:, t * VBLK : (t + 1) * VBLK],
                        start=(t == 0),
                        stop=(t == T - 1),
                    )
                blk = lg[:, vb * VBLK : (vb + 1) * VBLK]
                nc.vector.tensor_copy(blk, acc[:, :])
                mx8 = scratch[:, vb * K8 : (vb + 1) * K8]
                nc.vector.max(out=mx8, in_=blk)
                nc.vector.max_index(
                    out=idxs[:, vb * K8 : (vb + 1) * K8], in_max=mx8, in_values=blk
                )
            nc.sync.dma_start(out=outi.ap(), in_=idxs[:, :])

    nc.compile()
    return nc


TU = T // 2  # 16 contraction chunk-pairs for DoubleRow (K=256 each)
VPAD = 256  # moving-operand v stride (16-aligned padding of VBLK)


def _build_fp8dr():
    """fp8 DoubleRow prescreen: K=256 per PE pass, halving the column stream."""
    from concourse import bacc
    import concourse.mybir as mybir
    from concourse.tile import TileContext

    f32 = mybir.dt.float32
    f8 = mybir.dt.float8e4
    nc = bacc.Bacc("TRN2", debug=False, num_devices=NCORES)
    # wt[vb, p, u*2*VPAD + ko*VPAD + v] = W_shard[vb*VBLK + v, u*256 + ko*128 + p]
    wt = nc.dram_tensor("wt", [NVB, P, TU * 2 * VPAD], f8, kind="ExternalInput")
    # xt[p, u*32 + ko*16 + b] = x[b, u*256 + ko*128 + p] (slots b>=8 zero)
    xt_d = nc.dram_tensor("xt", [P, TU * 32], f32, kind="ExternalInput")
    gt_d = nc.dram_tensor("gt", [P, T], f32, kind="ExternalInput")
    outi = nc.dram_tensor("outi", [B, NVB * K8], mybir.dt.uint32, kind="ExternalOutput")

    with TileContext(nc) as tc:
        with (
            tc.tile_pool(name="const", bufs=1) as cpool,
            tc.tile_pool(name="wpool", bufs=8) as wpool,
            tc.tile_pool(name="psacc", bufs=3, space="PSUM") as psacc,
        ):
            xt = cpool.tile([P, TU * 32], f32)
            nc.gpsimd.dma_start(out=xt[:, :], in_=xt_d.ap())
            gt = cpool.tile([P, T], f32)
            nc.gpsimd.dma_start(out=gt[:, :], in_=gt_d.ap())
            hT = cpool.tile([P, TU * 32], f8)
            # one DVE op: hT[p, (u, ko, slot)] = xt * gt[p, (u, ko)] bcast over slot
            gt3 = gt[:, :].rearrange("p (u ko) -> p u ko", ko=2)
            nc.vector.tensor_tensor(
                out=hT[:, :].rearrange("p (u ko s) -> p u ko s", ko=2, s=16),
                in0=xt[:, :].rearrange("p (u ko s) -> p u ko s", ko=2, s=16),
                in1=gt3.to_broadcast([P, TU, 2, 16]),
                op=mybir.AluOpType.mult,
            )

            scratch = cpool.tile([B, NVB * K8], f32)
            idxs = cpool.tile([B, NVB * K8], mybir.dt.uint32)
            lg = cpool.tile([B, NVB * VBLK], f32)
            UH = TU // 2 * 2 * VPAD  # halfway point in the free dim
            for vb in range(NVB):
                w = wpool.tile([P, TU * 2 * VPAD], f8)
                # split each block across both HWDGE rings for 2x arrival rate
                nc.sync.dma_start(out=w[:, :UH], in_=wt.ap()[vb, :, :UH])
                nc.scalar.dma_start(out=w[:, UH:], in_=wt.ap()[vb, :, UH:])
                acc = psacc.tile([B, VBLK], f32)
                for u in range(TU):
                    lhs3 = hT[:, u * 32 : (u + 1) * 32].rearrange(
                        "p (ko b) -> p ko b", ko=2
                    )[:, :, :B]
                    rhs3 = w[:, u * 2 * VPAD : (u + 1) * 2 * VPAD].rearrange(
                        "p (ko v) -> p ko v", ko=2
                    )[:, :, :VBLK]
                    nc.tensor.matmul(
                        acc[:, :],
                        lhsT=lhs3,
                        rhs=rhs3,
                        start=(u == 0),
                        stop=(u == TU - 1),
                        perf_mode=mybir.MatmulPerfMode.DoubleRow,
                    )
                blk = lg[:, vb * VBLK : (vb + 1) * VBLK]
                nc.vector.tensor_copy(blk, acc[:, :])
                mx8 = scratch[:, vb * K8 : (vb + 1) * K8]
                nc.vector.max(out=mx8, in_=blk)
                nc.vector.max_index(
                    out=idxs[:, vb * K8 : (vb + 1) * K8], in_max=mx8, in_values=blk
                )
            nc.sync.dma_start(out=outi.ap(), in_=idxs[:, :])

    nc.compile()
    return nc


def _build_fp32():
    """Exact fp32 kernel (fallback): per-core global top-1 via (max, index)."""
    from concourse import bacc
    import concourse.mybir as mybir
    from concourse.tile import TileContext
    from concourse.masks import make_identity

    f32 = mybir.dt.float32
    NBANK, JCOL, VB = 8, 512, VS // 8
    nc = bacc.Bacc("TRN2", debug=False, num_devices=NCORES)
    wt = nc.dram_tensor("wt", [D, VS], f32, kind="ExternalInput")
    x = nc.dram_tensor("x", [B, D], f32, kind="ExternalInput")
    gt_d = nc.dram_tensor("gt", [P, T], f32, kind="ExternalInput")
    outv = nc.dram_tensor("outv", [B, 8], f32, kind="ExternalOutput")
    outi = nc.dram_tensor("outi", [B, 8], mybir.dt.uint32, kind="ExternalOutput")

    with TileContext(nc) as tc:
        with (
            tc.tile_pool(name="const", bufs=1) as cpool,
            tc.tile_pool(name="wpool", bufs=4) as wpool,
            tc.tile_pool(name="ps", bufs=1, space="PSUM") as pspool,
        ):
            xs = cpool.tile([B, D], f32)
            nc.gpsimd.dma_start(out=xs[:, :], in_=x.ap())
            gt = cpool.tile([P, T], f32)
            nc.gpsimd.dma_start(out=gt[:, :], in_=gt_d.ap())
            id8 = cpool.tile([B, B], f32)
            make_identity(nc, id8[:, :])

            xt = pspool.tile([P, T * B], f32, tag="ps")
            for t in range(T):
                nc.tensor.transpose(
                    out=xt[:, t * B : (t + 1) * B],
                    in_=xs[:, t * P : (t + 1) * P],
                    identity=id8[:, :],
                )
            hT = cpool.tile([P, T * B], f32)
            for t in range(T):
                nc.vector.tensor_scalar_mul(
                    hT[:, t * B : (t + 1) * B],
                    xt[:, t * B : (t + 1) * B],
                    gt[:, t : t + 1],
                )

            acc = pspool.tile([B, NBANK * JCOL], f32, tag="ps")
            for t in range(T):
                w = wpool.tile([P, VS], f32)
                dma_eng = nc.sync if t % 2 == 0 else nc.scalar
                dma_eng.dma_start(out=w[:, :], in_=wt.ap()[t * P : (t + 1) * P, :])
                for j in range(NBANK):
                    nc.tensor.matmul(
                        acc[:, j * JCOL : j * JCOL + VB],
                        lhsT=hT[:, t * B : (t + 1) * B],
                        rhs=w[:, j * VB : (j + 1) * VB],
                        start=(t == 0),
                        stop=(t == T - 1),
                    )

            logits = cpool.tile([B, VS], f32)
            for j in range(NBANK):
                nc.vector.tensor_copy(
                    logits[:, j * VB : (j + 1) * VB],
                    acc[:, j * JCOL : j * JCOL + VB],
                )
            mx = cpool.tile([B, 8], f32)
            mi = cpool.tile([B, 8], mybir.dt.uint32)
            nc.vector.max(out=mx[:, :], in_=logits[:, :])
            nc.vector.max_index(out=mi[:, :], in_max=mx[:, :], in_values=logits[:, :])
            nc.sync.dma_start(out=outv.ap(), in_=mx[:, :])
            nc.sync.dma_start(out=outi.ap(), in_=mi[:, :])

    nc.compile()
    return nc


def _get_nc(mode):
    key = f"nc_{mode}"
    if key not in _STATE:
        _ensure_profile_hook()
        if mode == "fp16":
            _STATE[key] = _build_prescreen("float16")
        elif mode == "fp8":
            _STATE[key] = _build_prescreen("float8e4")
        elif mode == "fp8dr":
            _STATE[key] = _build_fp8dr()
        else:
            _STATE[key] = _build_fp32()
    return _STATE[key]


def _prep_common(hidden_states, norm_weight):
    x = np.ascontiguousarray(np.asarray(hidden_states, dtype=np.float32))
    g = np.asarray(norm_weight, dtype=np.float32).reshape(-1)
    gt = np.ascontiguousarray(g.reshape(T, P).T)  # gt[p, t] = gamma[t*128 + p]
    return x, g, gt


W_PRESCALE = 64.0  # lifts fp8 weights into the e4m3 normal range; argmax-invariant


def _prep_in_maps_prescreen(x, gt, lm_head_weight, W, mode):
    import concourse.mybir as mybir

    wt_key = (mode, id(lm_head_weight), W.shape)
    if _STATE.get("wt_key") != wt_key:
        if mode == "fp16":
            Wc = W.astype(np.float16)
        else:
            Wc = (W * np.float32(W_PRESCALE)).astype(mybir.dt.np(mybir.dt.float8e4))
        # wt[c, vb, p, t, v] = W[c*VS + vb*VBLK + v, t*P + p]
        W6 = Wc.reshape(NCORES, NVB, VBLK, T, P).transpose(0, 1, 4, 3, 2)
        _STATE["wt_all"] = np.ascontiguousarray(W6).reshape(NCORES, NVB, P, T * VBLK)
        _STATE["wt_key"] = wt_key
    wt_all = _STATE["wt_all"]
    # xt[p, t*B + b] = x[b, t*P + p] - layout-only transform
    xt = np.ascontiguousarray(x.T.reshape(T, P, B).transpose(1, 0, 2)).reshape(
        P, T * B
    )
    return [{"wt": wt_all[c], "xt": xt, "gt": gt} for c in range(NCORES)]


def _prep_in_maps_fp8dr(x, gt, lm_head_weight, W):
    import concourse.mybir as mybir

    e4m3 = mybir.dt.np(mybir.dt.float8e4)
    wt_key = ("fp8dr", id(lm_head_weight), W.shape)
    if _STATE.get("wt_key") != wt_key:
        W8 = (W * np.float32(W_PRESCALE)).astype(e4m3)
        # [c, vb, v, u, ko, p] -> [c, vb, p, u, ko, v(pad 256)]
        W6 = W8.reshape(NCORES, NVB, VBLK, TU, 2, P).transpose(0, 1, 5, 3, 4, 2)
        wt_all = np.zeros((NCORES, NVB, P, TU, 2, VPAD), dtype=e4m3)
        wt_all[..., :VBLK] = W6
        _STATE["wt_all"] = wt_all.reshape(NCORES, NVB, P, TU * 2 * VPAD)
        _STATE["wt_key"] = wt_key
    wt_all = _STATE["wt_all"]
    # xt[p, u*32 + ko*16 + b] = x[b, (2u+ko)*P + p], b-slots 8..15 zero
    xtb = x.T.reshape(T, P, B).transpose(1, 0, 2)  # [p, t, b]
    xt = np.zeros((P, TU, 2, 16), dtype=np.float32)
    xt[:, :, :, :B] = xtb.reshape(P, TU, 2, B)
    xt = np.ascontiguousarray(xt).reshape(P, TU * 32)
    return [{"wt": wt_all[c], "xt": xt, "gt": gt} for c in range(NCORES)]


def _prep_in_maps_fp32(x, gt, lm_head_weight, W):
    wt_key = ("fp32", id(lm_head_weight), W.shape)
    if _STATE.get("wt_key") != wt_key:
        W3 = W.reshape(NCORES, VS, D)
        _STATE["wt_all"] = np.ascontiguousarray(W3.transpose(0, 2, 1))
        _STATE["wt_key"] = wt_key
    wt_all = _STATE["wt_all"]
    return [{"wt": wt_all[c], "x": x, "gt": gt} for c in range(NCORES)]


def _prep_in_maps(hidden_states, norm_weight, lm_head_weight, mode=None):
    mode = mode or DEFAULT_MODE
    x, g, gt = _prep_common(hidden_states, norm_weight)
    W = np.asarray(lm_head_weight, dtype=np.float32)
    _STATE["h64"] = x.astype(np.float64) * g.astype(np.float64)  # for rescoring
    _STATE["W"] = W
    if mode in ("fp16", "fp8"):
        return _prep_in_maps_prescreen(x, gt, lm_head_weight, W, mode)
    if mode == "fp8dr":
        return _prep_in_maps_fp8dr(x, gt, lm_head_weight, W)
    return _prep_in_maps_fp32(x, gt, lm_head_weight, W)


def _combine_fp16(results):
    """Rescore every per-block candidate in f64 and take the exact argmax."""
    W = _STATE["W"]
    h64 = _STATE["h64"]  # [B, D]
    # candidate global indices: [core, b, vb*8] -> per row a set of indices
    cand = np.empty((NCORES, B, NVB * K8), dtype=np.int64)
    for c in range(NCORES):
        li = results[c]["outi"].astype(np.int64)  # [B, NVB*K8], local within block
        vb_base = np.repeat(np.arange(NVB, dtype=np.int64) * VBLK, K8)[None, :]
        cand[c] = li + vb_base + c * VS
    cand = cand.transpose(1, 0, 2).reshape(B, NCORES * NVB * K8)  # [B, ncand]
    token = np.empty((B, 1), dtype=np.int32)
    for b in range(B):
        idx = np.unique(cand[b])
        scores = W[idx].astype(np.float64) @ h64[b]
        smax = scores.max()
        token[b, 0] = idx[scores == smax].min()
    return token


def _combine_fp32(results):
    vals = np.stack([results[c]["outv"][:, 0] for c in range(NCORES)], axis=0)
    idxs = np.stack(
        [results[c]["outi"][:, 0].astype(np.int64) for c in range(NCORES)], axis=0
    )
    glob = idxs + (np.arange(NCORES, dtype=np.int64) * VS)[:, None]
    token = np.empty((B, 1), dtype=np.int32)
    for b in range(B):
        vmax = vals[:, b].max()
        cand = np.nonzero(vals[:, b] == vmax)[0]
        token[b, 0] = glob[cand, b].min()
    return token


def _combine(results, mode=None):
    mode = mode or DEFAULT_MODE
    if mode in ("fp16", "fp8", "fp8dr"):
        return _combine_fp16(results)
    return _combine_fp32(results)


def _run(in_maps, mode=None, trace=False, tmpdir=None):
    from concourse import bass_utils

    mode = mode or DEFAULT_MODE
    nc = _get_nc(mode)
    return bass_utils.run_bass_kernel_spmd(
        nc, in_maps, core_ids=list(range(NCORES)), trace=trace, tmpdir=tmpdir
    )


def kernel(hidden_states, norm_weight, lm_head_weight):
    mode = DEFAULT_MODE
    in_maps = _prep_in_maps(hidden_states, norm_weight, lm_head_weight, mode)
    res = _run(in_maps, mode)
    return _combine(res.results, mode)


# revision 20
# speedup vs baseline: 1.0140x; 1.0140x over previous
"""LmHead (RMSNorm -> vocab projection -> top-1 token) on 8 trn2 NeuronCores.

Sharding: lm_head_weight is split over the vocab dim (4000 rows per core,
tensor-parallel).  Each core streams its weight shard from HBM, computes
local logits for all 8 batch rows on the PE, and reduces them to local
top-8 candidate sets with the DVE Max8 unit.  The host then combines the
per-core candidates into the global argmax.

The kernel is memory-bound (weight streaming dominates), so the shard is
prepared host-side in the exact SBUF image the kernel wants:
  - transposed so the contraction dim D lands on SBUF partitions
    (16 KB-contiguous DMA descriptors, full HBM rate), and
  - cast to fp16, halving the bytes streamed and making each PE matmul a
    single pass (fp32 matmuls lower to two PE passes on trn2).
fp16 logits are a prescreen only: each core keeps the top-8 of every
250-column block (fp16 noise is ~5e-4 vs ~0.1 typical top-2 gaps, so the
true winner is always captured), and the host rescores all candidates
against the fp32 weights in float64 to pick the exact argmax.
"""

import os
import sys
import types

import numpy as np

B = 8
D = 4096
V = 32000
NCORES = 8
VS = V // NCORES  # 4000 vocab rows per core
P = 128
T = D // P  # 32 contraction chunks
NVB = 16  # vocab blocks per core
VBLK = VS // NVB  # 250 columns per block
K8 = 8  # Max8 width

DEFAULT_MODE = os.environ.get("LMHEAD_MODE", "fp16")

_STATE = {}


def _ensure_profile_hook():
    """Register the axon NTFF profiling hook if the image's antenv lacks it.

    Harmless when tracing is never requested; lets test.py pass trace=True.
    """
    if "antenv.axon_hooks" in sys.modules:
        return
    try:
        import antenv  # noqa: F401
        from trn_agent_boot.trn_boot import _ntff_profile_via_ctypes

        hook = _ntff_profile_via_ctypes("/opt/axon/libaxon_pjrt.so")
        mod = types.ModuleType("antenv.axon_hooks")
        mod.get_axon_ntff_profile_hook = lambda: hook
        mod.set_axon_ntff_profile_hook = lambda h: None
        sys.modules["antenv.axon_hooks"] = mod
    except Exception:
        pass


def _build_prescreen(wdt_name):
    """Reduced-precision prescreen kernel: per-block top-8 indices for host
    rescoring.  wdt_name: 'float16' or 'float8e4'."""
    from concourse import bacc
    import concourse.mybir as mybir
    from concourse.tile import TileContext
    from concourse.masks import make_identity

    f32 = mybir.dt.float32
    f16 = getattr(mybir.dt, wdt_name)
    nc = bacc.Bacc("TRN2", debug=False, num_devices=NCORES)
    # host layout: wt[vb, p, t, v] = W_shard[vb*VBLK + v, t*P + p], fp16/fp8
    wt = nc.dram_tensor("wt", [NVB, P, T * VBLK], f16, kind="ExternalInput")
    # host layout: xt[p, t, b] = x[b, t*P + p] (pure layout prep, no arithmetic)
    xt_d = nc.dram_tensor("xt", [P, T * B], f32, kind="ExternalInput")
    gt_d = nc.dram_tensor("gt", [P, T], f32, kind="ExternalInput")
    outi = nc.dram_tensor("outi", [B, NVB * K8], mybir.dt.uint32, kind="ExternalOutput")

    with TileContext(nc) as tc:
        with (
            tc.tile_pool(name="const", bufs=1) as cpool,
            tc.tile_pool(name="wpool", bufs=8) as wpool,
            tc.tile_pool(name="psacc", bufs=3, space="PSUM") as psacc,
        ):
            # --- Phase 0: hT[d, (t,b)] = cast(xT[d, (t,b)] * gamma[d-chunk t]) ---
            xt = cpool.tile([P, T * B], f32)
            nc.gpsimd.dma_start(out=xt[:, :], in_=xt_d.ap())
            gt = cpool.tile([P, T], f32)
            nc.gpsimd.dma_start(out=gt[:, :], in_=gt_d.ap())
            hT = cpool.tile([P, T * B], f16)
            for t in range(T):
                nc.vector.tensor_scalar_mul(
                    hT[:, t * B : (t + 1) * B],
                    xt[:, t * B : (t + 1) * B],
                    gt[:, t : t + 1],
                )

            # --- Phase 1: per vocab block, stream weights + matmul + local top-8 ---
            scratch = cpool.tile([B, NVB * K8], f32)  # per-block top-8 values
            idxs = cpool.tile([B, NVB * K8], mybir.dt.uint32)
            lg = cpool.tile([B, NVB * VBLK], f32)  # block logits (SBUF, for Max8)
            TH = T // 2
            for vb in range(NVB):
                w = wpool.tile([P, T * VBLK], f16)
                # split per-block stream across both HWDGE rings
                nc.sync.dma_start(
                    out=w[:, : TH * VBLK], in_=wt.ap()[vb, :, : TH * VBLK]
                )
                nc.scalar.dma_start(
                    out=w[:, TH * VBLK :], in_=wt.ap()[vb, :, TH * VBLK :]
                )
                acc = psacc.tile([B, VBLK], f32)
                for t in range(T):
                    nc.tensor.matmul(
                        acc[:, :],
                        lhsT=hT[:, t * B : (t + 1) * B],
                        rhs=w[:, t * VBLK : (t + 1) * VBLK],
                        start=(t == 0),
                        stop=(t == T - 1),
                    )
                blk = lg[:, vb * VBLK : (vb + 1) * VBLK]
                nc.vector.tensor_copy(blk, acc[:, :])
                mx8 = scratch[:, vb * K8 : (vb + 1) * K8]
                nc.vector.max(out=mx8, in_=blk)
                nc.vector.max_index(
                    out=idxs[:, vb * K8 : (vb + 1) * K8], in_max=mx8, in_values=blk
                )
            nc.sync.dma_start(out=outi.ap(), in_=idxs[:, :])

    nc.compile()
    return nc


TU = T // 2  # 16 contraction chunk-pairs for DoubleRow (K=256 each)
VPAD = 256  # moving-operand v stride (16-aligned padding of VBLK)


def _build_fp8dr():
    """fp8 DoubleRow prescreen: K=256 per PE pass, halving the column stream."""
    from concourse import bacc
    import concourse.mybir as mybir
    from concourse.tile import TileContext

    f32 = mybir.dt.float32
    f8 = mybir.dt.float8e4
    nc = bacc.Bacc("TRN2", debug=False, num_devices=NCORES)
    # wt[vb, p, u*2*VPAD + ko*VPAD + v] = W_shard[vb*VBLK + v, u*256 + ko*128 + p]
    wt = nc.dram_tensor("wt", [NVB, P, TU * 2 * VPAD], f8, kind="ExternalInput")
    # xt[p, u*32 + ko*16 + b] = x[b, u*256 + ko*128 + p] (slots b>=8 zero)
    xt_d = nc.dram_tensor("xt", [P, TU * 32], f32, kind="ExternalInput")
    gt_d = nc.dram_tensor("gt", [P, T], f32, kind="ExternalInput")
    outi = nc.dram_tensor("outi", [B, NVB * K8], mybir.dt.uint32, kind="ExternalOutput")

    with TileContext(nc) as tc:
        with (
            tc.tile_pool(name="const", bufs=1) as cpool,
            tc.tile_pool(name="wpool", bufs=8) as wpool,
            tc.tile_pool(name="psacc", bufs=3, space="PSUM") as psacc,
        ):
            xt = cpool.tile([P, TU * 32], f32)
            nc.gpsimd.dma_start(out=xt[:, :], in_=xt_d.ap())
            gt = cpool.tile([P, T], f32)
            nc.gpsimd.dma_start(out=gt[:, :], in_=gt_d.ap())
            hT = cpool.tile([P, TU * 32], f8)
            # one DVE op: hT[p, (u, ko, slot)] = xt * gt[p, (u, ko)] bcast over slot
            gt3 = gt[:, :].rearrange("p (u ko) -> p u ko", ko=2)
            nc.vector.tensor_tensor(
                out=hT[:, :].rearrange("p (u ko s) -> p u ko s", ko=2, s=16),
                in0=xt[:, :].rearrange("p (u ko s) -> p u ko s", ko=2, s=16),
                in1=gt3.to_broadcast([P, TU, 2, 16]),
                op=mybir.AluOpType.mult,
            )

            scratch = cpool.tile([B, NVB * K8], f32)
            idxs = cpool.tile([B, NVB * K8], mybir.dt.uint32)
            lg = cpool.tile([B, NVB * VBLK], f32)
            UH = TU // 2 * 2 * VPAD  # halfway point in the free dim
            for vb in range(NVB):
                w = wpool.tile([P, TU * 2 * VPAD], f8)
                if vb < 2:
                    # first blocks: split across both rings so PE starts sooner
                    nc.sync.dma_start(out=w[:, :UH], in_=wt.ap()[vb, :, :UH])
                    nc.scalar.dma_start(out=w[:, UH:], in_=wt.ap()[vb, :, UH:])
                else:
                    dma_eng = nc.sync if vb % 2 == 0 else nc.scalar
                    dma_eng.dma_start(out=w[:, :], in_=wt.ap()[vb])
                acc = psacc.tile([B, VBLK], f32)
                for u in range(TU):
                    lhs3 = hT[:, u * 32 : (u + 1) * 32].rearrange(
                        "p (ko b) -> p ko b", ko=2
                    )[:, :, :B]
                    rhs3 = w[:, u * 2 * VPAD : (u + 1) * 2 * VPAD].rearrange(
                        "p (ko v) -> p ko v", ko=2
                    )[:, :, :VBLK]
                    nc.tensor.matmul(
                        acc[:, :],
                        lhsT=lhs3,
                        rhs=rhs3,
                        start=(u == 0),
                        stop=(u == TU - 1),
                        perf_mode=mybir.MatmulPerfMode.DoubleRow,
                    )
                blk = lg[:, vb * VBLK : (vb + 1) * VBLK]
                nc.vector.tensor_copy(blk, acc[:, :])
                mx8 = scratch[:, vb * K8 : (vb + 1) * K8]
                nc.vector.max(out=mx8, in_=blk)
                nc.vector.max_index(
                    out=idxs[:, vb * K8 : (vb + 1) * K8], in_max=mx8, in_values=blk
                )
            nc.sync.dma_start(out=outi.ap(), in_=idxs[:, :])

    nc.compile()
    return nc


def _build_fp32():
    """Exact fp32 kernel (fallback): per-core global top-1 via (max, index)."""
    from concourse import bacc
    import concourse.mybir as mybir
    from concourse.tile import TileContext
    from concourse.masks import make_identity

    f32 = mybir.dt.float32
    NBANK, JCOL, VB = 8, 512, VS // 8
    nc = bacc.Bacc("TRN2", debug=False, num_devices=NCORES)
    wt = nc.dram_tensor("wt", [D, VS], f32, kind="ExternalInput")
    x = nc.dram_tensor("x", [B, D], f32, kind="ExternalInput")
    gt_d = nc.dram_tensor("gt", [P, T], f32, kind="ExternalInput")
    outv = nc.dram_tensor("outv", [B, 8], f32, kind="ExternalOutput")
    outi = nc.dram_tensor("outi", [B, 8], mybir.dt.uint32, kind="ExternalOutput")

    with TileContext(nc) as tc:
        with (
            tc.tile_pool(name="const", bufs=1) as cpool,
            tc.tile_pool(name="wpool", bufs=4) as wpool,
            tc.tile_pool(name="ps", bufs=1, space="PSUM") as pspool,
        ):
            xs = cpool.tile([B, D], f32)
            nc.gpsimd.dma_start(out=xs[:, :], in_=x.ap())
            gt = cpool.tile([P, T], f32)
            nc.gpsimd.dma_start(out=gt[:, :], in_=gt_d.ap())
            id8 = cpool.tile([B, B], f32)
            make_identity(nc, id8[:, :])

            xt = pspool.tile([P, T * B], f32, tag="ps")
            for t in range(T):
                nc.tensor.transpose(
                    out=xt[:, t * B : (t + 1) * B],
                    in_=xs[:, t * P : (t + 1) * P],
                    identity=id8[:, :],
                )
            hT = cpool.tile([P, T * B], f32)
            for t in range(T):
                nc.vector.tensor_scalar_mul(
                    hT[:, t * B : (t + 1) * B],
                    xt[:, t * B : (t + 1) * B],
                    gt[:, t : t + 1],
                )

            acc = pspool.tile([B, NBANK * JCOL], f32, tag="ps")
            for t in range(T):
                w = wpool.tile([P, VS], f32)
                dma_eng = nc.sync if t % 2 == 0 else nc.scalar
                dma_eng.dma_start(out=w[:, :], in_=wt.ap()[t * P : (t + 1) * P, :])
                for j in range(NBANK):
                    nc.tensor.matmul(
                        acc[:, j * JCOL : j * JCOL + VB],
                        lhsT=hT[:, t * B : (t + 1) * B],
                        rhs=w[:, j * VB : (j + 1) * VB],
                        start=(t == 0),
                        stop=(t == T - 1),
                    )

            logits = cpool.tile([B, VS], f32)
            for j in range(NBANK):
                nc.vector.tensor_copy(
                    logits[:, j * VB : (j + 1) * VB],
                    acc[:, j * JCOL : j * JCOL + VB],
                )
            mx = cpool.tile([B, 8], f32)
            mi = cpool.tile([B, 8], mybir.dt.uint32)
            nc.vector.max(out=mx[:, :], in_=logits[:, :])
            nc.vector.max_index(out=mi[:, :], in_max=mx[:, :], in_values=logits[:, :])
            nc.sync.dma_start(out=outv.ap(), in_=mx[:, :])
            nc.sync.dma_start(out=outi.ap(), in_=mi[:, :])

    nc.compile()
    return nc


def _get_nc(mode):
    key = f"nc_{mode}"
    if key not in _STATE:
        _ensure_profile_hook()
        if mode == "fp16":
            _STATE[key] = _build_prescreen("float16")
        elif mode == "fp8":
            _STATE[key] = _build_prescreen("float8e4")
        elif mode == "fp8dr":
            _STATE[key] = _build_fp8dr()
        else:
            _STATE[key] = _build_fp32()
    return _STATE[key]


def _prep_common(hidden_states, norm_weight):
    x = np.ascontiguousarray(np.asarray(hidden_states, dtype=np.float32))
    g = np.asarray(norm_weight, dtype=np.float32).reshape(-1)
    gt = np.ascontiguousarray(g.reshape(T, P).T)  # gt[p, t] = gamma[t*128 + p]
    return x, g, gt


W_PRESCALE = 64.0  # lifts fp8 weights into the e4m3 normal range; argmax-invariant


def _prep_in_maps_prescreen(x, gt, lm_head_weight, W, mode):
    import concourse.mybir as mybir

    wt_key = (mode, id(lm_head_weight), W.shape)
    if _STATE.get("wt_key") != wt_key:
        if mode == "fp16":
            Wc = W.astype(np.float16)
        else:
            Wc = (W * np.float32(W_PRESCALE)).astype(mybir.dt.np(mybir.dt.float8e4))
        # wt[c, vb, p, t, v] = W[c*VS + vb*VBLK + v, t*P + p]
        W6 = Wc.reshape(NCORES, NVB, VBLK, T, P).transpose(0, 1, 4, 3, 2)
        _STATE["wt_all"] = np.ascontiguousarray(W6).reshape(NCORES, NVB, P, T * VBLK)
        _STATE["wt_key"] = wt_key
    wt_all = _STATE["wt_all"]
    # xt[p, t*B + b] = x[b, t*P + p] - layout-only transform
    xt = np.ascontiguousarray(x.T.reshape(T, P, B).transpose(1, 0, 2)).reshape(
        P, T * B
    )
    return [{"wt": wt_all[c], "xt": xt, "gt": gt} for c in range(NCORES)]


def _prep_in_maps_fp8dr(x, gt, lm_head_weight, W):
    import concourse.mybir as mybir

    e4m3 = mybir.dt.np(mybir.dt.float8e4)
    wt_key = ("fp8dr", id(lm_head_weight), W.shape)
    if _STATE.get("wt_key") != wt_key:
        W8 = (W * np.float32(W_PRESCALE)).astype(e4m3)
        # [c, vb, v, u, ko, p] -> [c, vb, p, u, ko, v(pad 256)]
        W6 = W8.reshape(NCORES, NVB, VBLK, TU, 2, P).transpose(0, 1, 5, 3, 4, 2)
        wt_all = np.zeros((NCORES, NVB, P, TU, 2, VPAD), dtype=e4m3)
        wt_all[..., :VBLK] = W6
        _STATE["wt_all"] = wt_all.reshape(NCORES, NVB, P, TU * 2 * VPAD)
        _STATE["wt_key"] = wt_key
    wt_all = _STATE["wt_all"]
    # xt[p, u*32 + ko*16 + b] = x[b, (2u+ko)*P + p], b-slots 8..15 zero
    xtb = x.T.reshape(T, P, B).transpose(1, 0, 2)  # [p, t, b]
    xt = np.zeros((P, TU, 2, 16), dtype=np.float32)
    xt[:, :, :, :B] = xtb.reshape(P, TU, 2, B)
    xt = np.ascontiguousarray(xt).reshape(P, TU * 32)
    return [{"wt": wt_all[c], "xt": xt, "gt": gt} for c in range(NCORES)]


def _prep_in_maps_fp32(x, gt, lm_head_weight, W):
    wt_key = ("fp32", id(lm_head_weight), W.shape)
    if _STATE.get("wt_key") != wt_key:
        W3 = W.reshape(NCORES, VS, D)
        _STATE["wt_all"] = np.ascontiguousarray(W3.transpose(0, 2, 1))
        _STATE["wt_key"] = wt_key
    wt_all = _STATE["wt_all"]
    return [{"wt": wt_all[c], "x": x, "gt": gt} for c in range(NCORES)]


def _prep_in_maps(hidden_states, norm_weight, lm_head_weight, mode=None):
    mode = mode or DEFAULT_MODE
    x, g, gt = _prep_common(hidden_states, norm_weight)
    W = np.asarray(lm_head_weight, dtype=np.float32)
    _STATE["h64"] = x.astype(np.float64) * g.astype(np.float64)  # for rescoring
    _STATE["W"] = W
    if mode in ("fp16", "fp8"):
        return _prep_in_maps_prescreen(x, gt, lm_head_weight, W, mode)
    if mode == "fp8dr":
        return _prep_in_maps_fp8dr(x, gt, lm_head_weight, W)
    return _prep_in_maps_fp32(x, gt, lm_head_weight, W)


def _combine_fp16(results):
    """Rescore every per-block candidate in f64 and take the exact argmax."""
    W = _STATE["W"]
    h64 = _STATE["h64"]  # [B, D]
    # candidate global indices: [core, b, vb*8] -> per row a set of indices
    cand = np.empty((NCORES, B, NVB * K8), dtype=np.int64)
    for c in range(NCORES):
        li = results[c]["outi"].astype(np.int64)  # [B, NVB*K8], local within block
        vb_base = np.repeat(np.arange(NVB, dtype=np.int64) * VBLK, K8)[None, :]
        cand[c] = li + vb_base + c * VS
    cand = cand.transpose(1, 0, 2).reshape(B, NCORES * NVB * K8)  # [B, ncand]
    token = np.empty((B, 1), dtype=np.int32)
    for b in range(B):
        idx = np.unique(cand[b])
        scores = W[idx].astype(np.float64) @ h64[b]
        smax = scores.max()
        token[b, 0] = idx[scores == smax].min()
    return token


def _combine_fp32(results):
    vals = np.stack([results[c]["outv"][:, 0] for c in range(NCORES)], axis=0)
    idxs = np.stack(
        [results[c]["outi"][:, 0].astype(np.int64) for c in range(NCORES)], axis=0
    )
    glob = idxs + (np.arange(NCORES, dtype=np.int64) * VS)[:, None]
    token = np.empty((B, 1), dtype=np.int32)
    for b in range(B):
        vmax = vals[:, b].max()
        cand = np.nonzero(vals[:, b] == vmax)[0]
        token[b, 0] = glob[cand, b].min()
    return token


def _combine(results, mode=None):
    mode = mode or DEFAULT_MODE
    if mode in ("fp16", "fp8", "fp8dr"):
        return _combine_fp16(results)
    return _combine_fp32(results)


def _run(in_maps, mode=None, trace=False, tmpdir=None):
    from concourse import bass_utils

    mode = mode or DEFAULT_MODE
    nc = _get_nc(mode)
    return bass_utils.run_bass_kernel_spmd(
        nc, in_maps, core_ids=list(range(NCORES)), trace=trace, tmpdir=tmpdir
    )


def kernel(hidden_states, norm_weight, lm_head_weight):
    mode = DEFAULT_MODE
    in_maps = _prep_in_maps(hidden_states, norm_weight, lm_head_weight, mode)
    res = _run(in_maps, mode)
    return _combine(res.results, mode)


# revision 21
# speedup vs baseline: 1.0498x; 1.0353x over previous
"""LmHead (RMSNorm -> vocab projection -> top-1 token) on 8 trn2 NeuronCores.

Sharding: lm_head_weight is split over the vocab dim (4000 rows per core,
tensor-parallel).  Each core streams its weight shard from HBM, computes
local logits for all 8 batch rows on the PE, and reduces them to local
top-8 candidate sets with the DVE Max8 unit.  The host then combines the
per-core candidates into the global argmax.

The kernel is memory-bound (weight streaming dominates), so the shard is
prepared host-side in the exact SBUF image the kernel wants:
  - transposed so the contraction dim D lands on SBUF partitions
    (16 KB-contiguous DMA descriptors, full HBM rate), and
  - cast to fp16, halving the bytes streamed and making each PE matmul a
    single pass (fp32 matmuls lower to two PE passes on trn2).
fp16 logits are a prescreen only: each core keeps the top-8 of every
250-column block (fp16 noise is ~5e-4 vs ~0.1 typical top-2 gaps, so the
true winner is always captured), and the host rescores all candidates
against the fp32 weights in float64 to pick the exact argmax.
"""

import os
import sys
import types

import numpy as np

B = 8
D = 4096
V = 32000
NCORES = 8
VS = V // NCORES  # 4000 vocab rows per core
P = 128
T = D // P  # 32 contraction chunks
NVB = 16  # vocab blocks per core
VBLK = VS // NVB  # 250 columns per block
K8 = 8  # Max8 width

DEFAULT_MODE = os.environ.get("LMHEAD_MODE", "fp16")

_STATE = {}


def _ensure_profile_hook():
    """Register the axon NTFF profiling hook if the image's antenv lacks it.

    Harmless when tracing is never requested; lets test.py pass trace=True.
    """
    if "antenv.axon_hooks" in sys.modules:
        return
    try:
        import antenv  # noqa: F401
        from trn_agent_boot.trn_boot import _ntff_profile_via_ctypes

        hook = _ntff_profile_via_ctypes("/opt/axon/libaxon_pjrt.so")
        mod = types.ModuleType("antenv.axon_hooks")
        mod.get_axon_ntff_profile_hook = lambda: hook
        mod.set_axon_ntff_profile_hook = lambda h: None
        sys.modules["antenv.axon_hooks"] = mod
    except Exception:
        pass


def _build_prescreen(wdt_name):
    """Reduced-precision prescreen kernel: per-block top-8 indices for host
    rescoring.  wdt_name: 'float16' or 'float8e4'."""
    from concourse import bacc
    import concourse.mybir as mybir
    from concourse.tile import TileContext
    from concourse.masks import make_identity

    f32 = mybir.dt.float32
    f16 = getattr(mybir.dt, wdt_name)
    nc = bacc.Bacc("TRN2", debug=False, num_devices=NCORES)
    # host layout: wt[vb, p, t, v] = W_shard[vb*VBLK + v, t*P + p], fp16/fp8
    wt = nc.dram_tensor("wt", [NVB, P, T * VBLK], f16, kind="ExternalInput")
    # host layout: xt[p, t, b] = x[b, t*P + p] (pure layout prep, no arithmetic)
    xt_d = nc.dram_tensor("xt", [P, T * B], f32, kind="ExternalInput")
    gt_d = nc.dram_tensor("gt", [P, T], f32, kind="ExternalInput")
    outi = nc.dram_tensor("outi", [B, NVB * K8], mybir.dt.uint32, kind="ExternalOutput")

    with TileContext(nc) as tc:
        with (
            tc.tile_pool(name="const", bufs=1) as cpool,
            tc.tile_pool(name="wpool", bufs=8) as wpool,
            tc.tile_pool(name="psacc", bufs=3, space="PSUM") as psacc,
        ):
            # --- Phase 0: hT[d, (t,b)] = cast(xT[d, (t,b)] * gamma[d-chunk t]) ---
            xt = cpool.tile([P, T * B], f32)
            nc.gpsimd.dma_start(out=xt[:, :], in_=xt_d.ap())
            gt = cpool.tile([P, T], f32)
            nc.gpsimd.dma_start(out=gt[:, :], in_=gt_d.ap())
            hT = cpool.tile([P, T * B], f16)
            for t in range(T):
                nc.vector.tensor_scalar_mul(
                    hT[:, t * B : (t + 1) * B],
                    xt[:, t * B : (t + 1) * B],
                    gt[:, t : t + 1],
                )

            # --- Phase 1: per vocab block, stream weights + matmul + local top-8 ---
            scratch = cpool.tile([B, NVB * K8], f32)  # per-block top-8 values
            idxs = cpool.tile([B, NVB * K8], mybir.dt.uint32)
            lg = cpool.tile([B, NVB * VBLK], f32)  # block logits (SBUF, for Max8)
            TH = T // 2
            for vb in range(NVB):
                w = wpool.tile([P, T * VBLK], f16)
                # split per-block stream across both HWDGE rings
                nc.sync.dma_start(
                    out=w[:, : TH * VBLK], in_=wt.ap()[vb, :, : TH * VBLK]
                )
                nc.scalar.dma_start(
                    out=w[:, TH * VBLK :], in_=wt.ap()[vb, :, TH * VBLK :]
                )
                acc = psacc.tile([B, VBLK], f32)
                for t in range(T):
                    nc.tensor.matmul(
                        acc[:, :],
                        lhsT=hT[:, t * B : (t + 1) * B],
                        rhs=w[:, t * VBLK : (t + 1) * VBLK],
                        start=(t == 0),
                        stop=(t == T - 1),
                    )
                blk = lg[:, vb * VBLK : (vb + 1) * VBLK]
                nc.vector.tensor_copy(blk, acc[:, :])
                mx8 = scratch[:, vb * K8 : (vb + 1) * K8]
                nc.vector.max(out=mx8, in_=blk)
                nc.vector.max_index(
                    out=idxs[:, vb * K8 : (vb + 1) * K8], in_max=mx8, in_values=blk
                )
            nc.sync.dma_start(out=outi.ap(), in_=idxs[:, :])

    nc.compile()
    return nc


TU = T // 2  # 16 contraction chunk-pairs for DoubleRow (K=256 each)
VPAD = 256  # moving-operand v stride (16-aligned padding of VBLK)


def _build_fp8dr():
    """fp8 DoubleRow prescreen: K=256 per PE pass, halving the column stream."""
    from concourse import bacc
    import concourse.mybir as mybir
    from concourse.tile import TileContext

    f32 = mybir.dt.float32
    f8 = mybir.dt.float8e4
    nc = bacc.Bacc("TRN2", debug=False, num_devices=NCORES)
    # wt[vb, p, u*2*VPAD + ko*VPAD + v] = W_shard[vb*VBLK + v, u*256 + ko*128 + p]
    wt = nc.dram_tensor("wt", [NVB, P, TU * 2 * VPAD], f8, kind="ExternalInput")
    # xt[p, u*32 + ko*16 + b] = x[b, u*256 + ko*128 + p] (slots b>=8 zero)
    xt_d = nc.dram_tensor("xt", [P, TU * 32], f32, kind="ExternalInput")
    gt_d = nc.dram_tensor("gt", [P, T], f32, kind="ExternalInput")
    outi = nc.dram_tensor("outi", [B, NVB * K8], mybir.dt.uint32, kind="ExternalOutput")

    with TileContext(nc) as tc:
        with (
            tc.tile_pool(name="const", bufs=1) as cpool,
            tc.tile_pool(name="wpool", bufs=8) as wpool,
            tc.tile_pool(name="psacc", bufs=3, space="PSUM") as psacc,
        ):
            xt = cpool.tile([P, TU * 32], f32)
            nc.gpsimd.dma_start(out=xt[:, :], in_=xt_d.ap())
            gt = cpool.tile([P, T], f32)
            nc.gpsimd.dma_start(out=gt[:, :], in_=gt_d.ap())
            hT = cpool.tile([P, TU * 32], f8)
            for u in range(TU):
                for ko in range(2):
                    s = u * 32 + ko * 16
                    nc.vector.tensor_scalar_mul(
                        hT[:, s : s + 8],
                        xt[:, s : s + 8],
                        gt[:, 2 * u + ko : 2 * u + ko + 1],
                    )

            scratch = cpool.tile([B, NVB * K8], f32)
            idxs = cpool.tile([B, NVB * K8], mybir.dt.uint32)
            lg = cpool.tile([B, NVB * VBLK], f32)
            UH = TU // 2 * 2 * VPAD  # halfway point in the free dim
            for vb in range(NVB):
                w = wpool.tile([P, TU * 2 * VPAD], f8)
                if vb < 2:
                    # first blocks: split across both rings so PE starts sooner
                    nc.sync.dma_start(out=w[:, :UH], in_=wt.ap()[vb, :, :UH])
                    nc.scalar.dma_start(out=w[:, UH:], in_=wt.ap()[vb, :, UH:])
                else:
                    dma_eng = nc.sync if vb % 2 == 0 else nc.scalar
                    dma_eng.dma_start(out=w[:, :], in_=wt.ap()[vb])
                acc = psacc.tile([B, VBLK], f32)
                for u in range(TU):
                    lhs3 = hT[:, u * 32 : (u + 1) * 32].rearrange(
                        "p (ko b) -> p ko b", ko=2
                    )[:, :, :B]
                    rhs3 = w[:, u * 2 * VPAD : (u + 1) * 2 * VPAD].rearrange(
                        "p (ko v) -> p ko v", ko=2
                    )[:, :, :VBLK]
                    nc.tensor.matmul(
                        acc[:, :],
                        lhsT=lhs3,
                        rhs=rhs3,
                        start=(u == 0),
                        stop=(u == TU - 1),
                        perf_mode=mybir.MatmulPerfMode.DoubleRow,
                    )
                blk = lg[:, vb * VBLK : (vb + 1) * VBLK]
                nc.vector.tensor_copy(blk, acc[:, :])
                mx8 = scratch[:, vb * K8 : (vb + 1) * K8]
                nc.vector.max(out=mx8, in_=blk)
                nc.vector.max_index(
                    out=idxs[:, vb * K8 : (vb + 1) * K8], in_max=mx8, in_values=blk
                )
            nc.sync.dma_start(out=outi.ap(), in_=idxs[:, :])

    nc.compile()
    return nc


def _build_fp32():
    """Exact fp32 kernel (fallback): per-core global top-1 via (max, index)."""
    from concourse import bacc
    import concourse.mybir as mybir
    from concourse.tile import TileContext
    from concourse.masks import make_identity

    f32 = mybir.dt.float32
    NBANK, JCOL, VB = 8, 512, VS // 8
    nc = bacc.Bacc("TRN2", debug=False, num_devices=NCORES)
    wt = nc.dram_tensor("wt", [D, VS], f32, kind="ExternalInput")
    x = nc.dram_tensor("x", [B, D], f32, kind="ExternalInput")
    gt_d = nc.dram_tensor("gt", [P, T], f32, kind="ExternalInput")
    outv = nc.dram_tensor("outv", [B, 8], f32, kind="ExternalOutput")
    outi = nc.dram_tensor("outi", [B, 8], mybir.dt.uint32, kind="ExternalOutput")

    with TileContext(nc) as tc:
        with (
            tc.tile_pool(name="const", bufs=1) as cpool,
            tc.tile_pool(name="wpool", bufs=4) as wpool,
            tc.tile_pool(name="ps", bufs=1, space="PSUM") as pspool,
        ):
            xs = cpool.tile([B, D], f32)
            nc.gpsimd.dma_start(out=xs[:, :], in_=x.ap())
            gt = cpool.tile([P, T], f32)
            nc.gpsimd.dma_start(out=gt[:, :], in_=gt_d.ap())
            id8 = cpool.tile([B, B], f32)
            make_identity(nc, id8[:, :])

            xt = pspool.tile([P, T * B], f32, tag="ps")
            for t in range(T):
                nc.tensor.transpose(
                    out=xt[:, t * B : (t + 1) * B],
                    in_=xs[:, t * P : (t + 1) * P],
                    identity=id8[:, :],
                )
            hT = cpool.tile([P, T * B], f32)
            for t in range(T):
                nc.vector.tensor_scalar_mul(
                    hT[:, t * B : (t + 1) * B],
                    xt[:, t * B : (t + 1) * B],
                    gt[:, t : t + 1],
                )

            acc = pspool.tile([B, NBANK * JCOL], f32, tag="ps")
            for t in range(T):
                w = wpool.tile([P, VS], f32)
                dma_eng = nc.sync if t % 2 == 0 else nc.scalar
                dma_eng.dma_start(out=w[:, :], in_=wt.ap()[t * P : (t + 1) * P, :])
                for j in range(NBANK):
                    nc.tensor.matmul(
                        acc[:, j * JCOL : j * JCOL + VB],
                        lhsT=hT[:, t * B : (t + 1) * B],
                        rhs=w[:, j * VB : (j + 1) * VB],
                        start=(t == 0),
                        stop=(t == T - 1),
                    )

            logits = cpool.tile([B, VS], f32)
            for j in range(NBANK):
                nc.vector.tensor_copy(
                    logits[:, j * VB : (j + 1) * VB],
                    acc[:, j * JCOL : j * JCOL + VB],
                )
            mx = cpool.tile([B, 8], f32)
            mi = cpool.tile([B, 8], mybir.dt.uint32)
            nc.vector.max(out=mx[:, :], in_=logits[:, :])
            nc.vector.max_index(out=mi[:, :], in_max=mx[:, :], in_values=logits[:, :])
            nc.sync.dma_start(out=outv.ap(), in_=mx[:, :])
            nc.sync.dma_start(out=outi.ap(), in_=mi[:, :])

    nc.compile()
    return nc


def _get_nc(mode):
    key = f"nc_{mode}"
    if key not in _STATE:
        _ensure_profile_hook()
        if mode == "fp16":
            _STATE[key] = _build_prescreen("float16")
        elif mode == "fp8":
            _STATE[key] = _build_prescreen("float8e4")
        elif mode == "fp8dr":
            _STATE[key] = _build_fp8dr()
        else:
            _STATE[key] = _build_fp32()
    return _STATE[key]


def _prep_common(hidden_states, norm_weight):
    x = np.ascontiguousarray(np.asarray(hidden_states, dtype=np.float32))
    g = np.asarray(norm_weight, dtype=np.float32).reshape(-1)
    gt = np.ascontiguousarray(g.reshape(T, P).T)  # gt[p, t] = gamma[t*128 + p]
    return x, g, gt


W_PRESCALE = 64.0  # lifts fp8 weights into the e4m3 normal range; argmax-invariant


def _prep_in_maps_prescreen(x, gt, lm_head_weight, W, mode):
    import concourse.mybir as mybir

    wt_key = (mode, id(lm_head_weight), W.shape)
    if _STATE.get("wt_key") != wt_key:
        if mode == "fp16":
            Wc = W.astype(np.float16)
        else:
            Wc = (W * np.float32(W_PRESCALE)).astype(mybir.dt.np(mybir.dt.float8e4))
        # wt[c, vb, p, t, v] = W[c*VS + vb*VBLK + v, t*P + p]
        W6 = Wc.reshape(NCORES, NVB, VBLK, T, P).transpose(0, 1, 4, 3, 2)
        _STATE["wt_all"] = np.ascontiguousarray(W6).reshape(NCORES, NVB, P, T * VBLK)
        _STATE["wt_key"] = wt_key
    wt_all = _STATE["wt_all"]
    # xt[p, t*B + b] = x[b, t*P + p] - layout-only transform
    xt = np.ascontiguousarray(x.T.reshape(T, P, B).transpose(1, 0, 2)).reshape(
        P, T * B
    )
    return [{"wt": wt_all[c], "xt": xt, "gt": gt} for c in range(NCORES)]


def _prep_in_maps_fp8dr(x, gt, lm_head_weight, W):
    import concourse.mybir as mybir

    e4m3 = mybir.dt.np(mybir.dt.float8e4)
    wt_key = ("fp8dr", id(lm_head_weight), W.shape)
    if _STATE.get("wt_key") != wt_key:
        W8 = (W * np.float32(W_PRESCALE)).astype(e4m3)
        # [c, vb, v, u, ko, p] -> [c, vb, p, u, ko, v(pad 256)]
        W6 = W8.reshape(NCORES, NVB, VBLK, TU, 2, P).transpose(0, 1, 5, 3, 4, 2)
        wt_all = np.zeros((NCORES, NVB, P, TU, 2, VPAD), dtype=e4m3)
        wt_all[..., :VBLK] = W6
        _STATE["wt_all"] = wt_all.reshape(NCORES, NVB, P, TU * 2 * VPAD)
        _STATE["wt_key"] = wt_key
    wt_all = _STATE["wt_all"]
    # xt[p, u*32 + ko*16 + b] = x[b, (2u+ko)*P + p], b-slots 8..15 zero
    xtb = x.T.reshape(T, P, B).transpose(1, 0, 2)  # [p, t, b]
    xt = np.zeros((P, TU, 2, 16), dtype=np.float32)
    xt[:, :, :, :B] = xtb.reshape(P, TU, 2, B)
    xt = np.ascontiguousarray(xt).reshape(P, TU * 32)
    return [{"wt": wt_all[c], "xt": xt, "gt": gt} for c in range(NCORES)]


def _prep_in_maps_fp32(x, gt, lm_head_weight, W):
    wt_key = ("fp32", id(lm_head_weight), W.shape)
    if _STATE.get("wt_key") != wt_key:
        W3 = W.reshape(NCORES, VS, D)
        _STATE["wt_all"] = np.ascontiguousarray(W3.transpose(0, 2, 1))
        _STATE["wt_key"] = wt_key
    wt_all = _STATE["wt_all"]
    return [{"wt": wt_all[c], "x": x, "gt": gt} for c in range(NCORES)]


def _prep_in_maps(hidden_states, norm_weight, lm_head_weight, mode=None):
    mode = mode or DEFAULT_MODE
    x, g, gt = _prep_common(hidden_states, norm_weight)
    W = np.asarray(lm_head_weight, dtype=np.float32)
    _STATE["h64"] = x.astype(np.float64) * g.astype(np.float64)  # for rescoring
    _STATE["W"] = W
    if mode in ("fp16", "fp8"):
        return _prep_in_maps_prescreen(x, gt, lm_head_weight, W, mode)
    if mode == "fp8dr":
        return _prep_in_maps_fp8dr(x, gt, lm_head_weight, W)
    return _prep_in_maps_fp32(x, gt, lm_head_weight, W)


def _combine_fp16(results):
    """Rescore every per-block candidate in f64 and take the exact argmax."""
    W = _STATE["W"]
    h64 = _STATE["h64"]  # [B, D]
    # candidate global indices: [core, b, vb*8] -> per row a set of indices
    cand = np.empty((NCORES, B, NVB * K8), dtype=np.int64)
    for c in range(NCORES):
        li = results[c]["outi"].astype(np.int64)  # [B, NVB*K8], local within block
        vb_base = np.repeat(np.arange(NVB, dtype=np.int64) * VBLK, K8)[None, :]
        cand[c] = li + vb_base + c * VS
    cand = cand.transpose(1, 0, 2).reshape(B, NCORES * NVB * K8)  # [B, ncand]
    token = np.empty((B, 1), dtype=np.int32)
    for b in range(B):
        idx = np.unique(cand[b])
        scores = W[idx].astype(np.float64) @ h64[b]
        smax = scores.max()
        token[b, 0] = idx[scores == smax].min()
    return token


def _combine_fp32(results):
    vals = np.stack([results[c]["outv"][:, 0] for c in range(NCORES)], axis=0)
    idxs = np.stack(
        [results[c]["outi"][:, 0].astype(np.int64) for c in range(NCORES)], axis=0
    )
    glob = idxs + (np.arange(NCORES, dtype=np.int64) * VS)[:, None]
    token = np.empty((B, 1), dtype=np.int32)
    for b in range(B):
        vmax = vals[:, b].max()
        cand = np.nonzero(vals[:, b] == vmax)[0]
        token[b, 0] = glob[cand, b].min()
    return token


def _combine(results, mode=None):
    mode = mode or DEFAULT_MODE
    if mode in ("fp16", "fp8", "fp8dr"):
        return _combine_fp16(results)
    return _combine_fp32(results)


def _run(in_maps, mode=None, trace=False, tmpdir=None):
    from concourse import bass_utils

    mode = mode or DEFAULT_MODE
    nc = _get_nc(mode)
    return bass_utils.run_bass_kernel_spmd(
        nc, in_maps, core_ids=list(range(NCORES)), trace=trace, tmpdir=tmpdir
    )


def kernel(hidden_states, norm_weight, lm_head_weight):
    mode = DEFAULT_MODE
    in_maps = _prep_in_maps(hidden_states, norm_weight, lm_head_weight, mode)
    res = _run(in_maps, mode)
    return _combine(res.results, mode)
